# revision 40
# baseline (speedup 1.0000x reference)
"""TRN2 Bass kernel for nn_MaxRollingMeanAttentionProbe (sparse_attention).

Computation (reference):
    y      = relu(x @ w1 + b1)                    # [n, 256]
    logits = y @ queries.T ; vals = y @ values.T  # [n, 16]
    window i of size w: score_i = sum_j softmax(logits[i:i+w])_j * vals[i:i+w]_j
    out    = sum_h max_i score[i, h]              # scalar
Strategy: data-parallel over tokens across 8 NeuronCores with a recomputed
(w-1)-token halo, so no collectives are needed (the softmax shift cancels
exactly within any window).

Per core (one SPMD Tile program):
  pass A: stream host pre-packed fp8-e4m3 x tiles [128 dm, 16 chunks, 512 tok];
          DoubleRow fp8 matmuls (2 k-tiles per instruction, 157 TF/s) accumulate
          yT = relu(w1.T @ xT + b1) in fp32 PSUM; the combined fp8 [queries;
          values] lhsT (pre-scaled; descaled in pass B / on host) yields
          logits+vals stacked in one [32, g] PSUM tile via a single DoubleRow
          matmul; probe work is software-pipelined one group behind the MLP.
          Each group's probe output is evicted (DVE) and DMA-regrouped into the
          pass-B layout immediately, so pass B can start before pass A ends.
  pass B: layout [128 partitions = 8 subchunks x 16 heads, 2048 + w - 1
          tokens]; processed in 512-column chunks as their columns complete:
          exp on ScalarE (with probe descale); per-chunk standalone DVE prefix
          scans (windows never span chunk reads thanks to a w-1 column halo);
          shifted subtracts / score multiply split across DVE + GpSimd; fast
          approximate reciprocal; per-chunk max-reduce into a [128, 2*NCH]
          accumulator -> [128, 2] result.
Host: pack/cast inputs (fp8 for matmul operands, pre-scaled to dodge fp8
subnormals), final max/sum + probe descale (tiny).
"""

import numpy as np

# Problem constants (shapes are fixed by the problem spec).
N_TOKENS = 131072
D_MODEL = 2048
D_HID = 256
N_HEADS = 16
N_CORES = 8
P = 128                    # SBUF partitions
G = 512                    # tokens per matmul/DMA group
TPC = N_TOKENS // N_CORES  # window starts per core (16384)
GPC = TPC // G             # groups per core without halo (32)
NSUB = 8                   # subchunks per core in pass B
SUB = TPC // NSUB          # window starts per subchunk (2048)
GPS = SUB // G             # groups per subchunk (4)
NCH = SUB // G             # pass-B column chunks per subchunk span (4)
ND = D_MODEL // P          # 16 d_model chunks
NH2 = D_HID // P           # 2 hidden halves

SCALE_W = 128.0   # host pre-scale on w1 so fp8 values clear the subnormal range
QV_SCALE = 1024.0  # host pre-scale on [queries; values] for the same reason

_NC_CACHE = {}


def _round_fp32r(a: np.ndarray) -> np.ndarray:
    """Round-to-nearest-even to fp32r (11-bit mantissa), new array."""
    u = np.ascontiguousarray(a, dtype=np.float32).view(np.uint32)
    r = (u + np.uint32(0x800) + ((u >> np.uint32(12)) & np.uint32(1))) & np.uint32(
        0xFFFFF000
    )
    return r.view(np.float32)


def _build(w: int, mmdt: str = "f8dr"):
    import concourse.bacc as bacc
    import concourse.tile as tile
    from concourse import mybir
    from contextlib import ExitStack

    F32 = mybir.dt.float32
    # x/w1/probe dtype; "f8dr" = fp8-e4m3 with DoubleRow paired-k matmuls.
    MDT = {"f16": mybir.dt.float16, "bf16": mybir.dt.bfloat16,
           "f32r": mybir.dt.float32r, "f8dr": mybir.dt.float8e4}[mmdt]
    DR = mybir.MatmulPerfMode.DoubleRow if mmdt == "f8dr" else None
    QVS = QV_SCALE if DR is not None else 1.0
    AF = mybir.ActivationFunctionType
    AX = mybir.AxisListType
    ADD = mybir.AluOpType.add
    BYP = mybir.AluOpType.bypass

    NG = -(-(TPC + w - 1) // G)    # groups per core incl. halo
    SUBLEN = SUB + w - 1           # tokens per subchunk incl. halo
    SPLIT = SUB - w + 1            # starts < SPLIT are always-valid windows
    TW = (SUBLEN + 15) // 16 * 16  # padded pass-B tile width

    nc = bacc.Bacc(
        "TRN2",
        target_bir_lowering=False,
        debug=False,
        enable_asserts=False,
        num_devices=N_CORES,
    )
    xg = nc.dram_tensor("xg", [NG, P, ND, G], MDT, kind="ExternalInput")
    w1p = nc.dram_tensor("w1p", [P, ND, D_HID], MDT, kind="ExternalInput")
    b1p = nc.dram_tensor("b1p", [P, NH2], F32, kind="ExternalInput")
    # Combined probe weights: columns 0..15 = queries, 16..31 = values -> one
    # matmul (pair) yields logits/vals stacked in one PSUM tile.
    if DR is not None:
        qvp = nc.dram_tensor("qvp", [P, NH2, 2 * N_HEADS], MDT, kind="ExternalInput")
    else:
        qvp = nc.dram_tensor(
            "qvp", [P, NH2, 2 * N_HEADS], mybir.dt.float16, kind="ExternalInput"
        )
    res = nc.dram_tensor("res", [P, 2], F32, kind="ExternalOutput")

    with tile.TileContext(nc) as tc, ExitStack() as ctx:
        const = ctx.enter_context(tc.tile_pool(name="const", bufs=1))
        w1_sb = const.tile([P, ND, D_HID], MDT)
        b1_sb = const.tile([P, NH2], F32)
        qv_sb = const.tile([P, NH2, 2 * N_HEADS], qvp.dtype)

        # Persistent pass-B layout: partition s*16+h, free dim = token within
        # subchunk s (0..SUBLEN). Filled per group via SBUF->SBUF DMA.
        # Pass-B chunks over window-start ranges [a, e). Boundaries are chosen
        # so that, with block-major group processing, each chunk's columns
        # complete as early as possible and only the last (w-1)-wide chunk
        # gates on the trailing halo group.
        cb = sorted({0, G, 2 * G, max(SPLIT - G, 0), SPLIT, SUB})
        CHUNKS = [(a, e) for a, e in zip(cb, cb[1:]) if e > a]
        NCHK = len(CHUNKS)

        bp = ctx.enter_context(tc.tile_pool(name="bp", bufs=1))
        RL = bp.tile([P, TW], F32)
        RV = bp.tile([P, TW], F32)
        sm = bp.tile([P, 2 * NCHK], F32)

        xpool = ctx.enter_context(tc.tile_pool(name="xpool", bufs=6))
        ypool = ctx.enter_context(tc.tile_pool(name="ypool", bufs=3))
        stpool = ctx.enter_context(tc.tile_pool(name="stpool", bufs=3))
        pbpool = ctx.enter_context(tc.tile_pool(name="pbpool", bufs=2))
        psy = ctx.enter_context(tc.tile_pool(name="psy", bufs=4, space="PSUM"))
        pslv = ctx.enter_context(tc.tile_pool(name="pslv", bufs=4, space="PSUM"))

        nc.vector.memset(sm[:], -3.0e38)

        # ---------------- pass B chunk (emitted as soon as columns land) ----
        def emit_passb_chunk(c):
            a, e = CHUNKS[c]
            CW = e - a + w - 1               # columns read (incl. w-1 halo)
            ns = e - a                       # window starts in this chunk
            E = pbpool.tile([P, CW], F32, tag="E")
            nc.scalar.activation(E[:], RL[:, a : a + CW], AF.Exp, scale=1.0 / QVS)
            EV = pbpool.tile([P, CW], F32, tag="EV")
            nc.vector.tensor_mul(EV[:], E[:], RV[:, a : a + CW])
            csZ = pbpool.tile([P, CW + 1], F32, tag="csZ")
            nc.vector.memset(csZ[:, 0:1], 0.0)
            nc.vector.tensor_tensor_scan(
                out=csZ[:, 1 : 1 + CW], data0=E[:], data1=E[:],
                initial=0.0, op0=ADD, op1=BYP,
            )
            csW = pbpool.tile([P, CW + 1], F32, tag="csW")
            nc.vector.memset(csW[:, 0:1], 0.0)
            nc.vector.tensor_tensor_scan(
                out=csW[:, 1 : 1 + CW], data0=EV[:], data1=EV[:],
                initial=0.0, op0=ADD, op1=BYP,
            )
            Z = pbpool.tile([P, ns], F32, tag="Z")
            nc.vector.tensor_sub(Z[:], csZ[:, w : w + ns], csZ[:, 0:ns])
            Wn = pbpool.tile([P, ns], F32, tag="Wn")
            nc.vector.tensor_sub(Wn[:], csW[:, w : w + ns], csW[:, 0:ns])
            R = pbpool.tile([P, ns], F32, tag="R")
            nc.vector.reciprocal_approx_fast(out=R[:], in_=Z[:])
            S = pbpool.tile([P, ns], F32, tag="S")
            nc.vector.tensor_mul(S[:], Wn[:], R[:])
            # starts in [a, min(a+ns, SPLIT)) are class-0 (always valid);
            # the rest are class-1 (invalid on the last core's last subchunk).
            n0 = min(max(SPLIT - a, 0), ns)
            if n0 > 0:
                nc.vector.reduce_max(out=sm[:, c : c + 1], in_=S[:, 0:n0], axis=AX.X)
            if n0 < ns:
                nc.vector.reduce_max(
                    out=sm[:, NCHK + c : NCHK + c + 1], in_=S[:, n0:ns], axis=AX.X
                )

        # ---------------- pass A helpers ----------------
        def emit_probes(g, yt, gw, last=False):
            """Probe matmul + eviction + pass-B flush for group g (called
            during iteration g+1). The flush DMAs are issued from the Scalar
            queue (not Sync) so they never stall the x-load prefetch stream;
            the final groups' flushes use the by-then-idle Sync queue so the
            tail's flush issues run on two queues in parallel."""
            lvp = pslv.tile([2 * N_HEADS, gw], F32, tag="lvp")
            if DR is not None:
                nc.tensor.matmul(
                    lvp[:], qv_sb[:, :, :], yt[:, :, :],
                    start=True, stop=True, perf_mode=DR,
                )
            else:
                for hh in range(NH2):
                    nc.tensor.matmul(
                        lvp[:], qv_sb[:, hh, :], yt[:, hh, :],
                        start=(hh == 0), stop=(hh == NH2 - 1),
                    )
            st = stpool.tile([2 * N_HEADS, gw], F32, tag="st")
            nc.vector.tensor_copy(out=st[:], in_=lvp[:])
            s, b = g // GPS, g % GPS
            qe = nc.sync if last else nc.scalar
            if s < NSUB:
                r0 = s * N_HEADS
                col = b * G
                qe.dma_start(
                    out=RL[r0 : r0 + N_HEADS, col : col + G], in_=st[0:N_HEADS, :]
                )
                nc.scalar.dma_start(
                    out=RV[r0 : r0 + N_HEADS, col : col + G],
                    in_=st[N_HEADS : 2 * N_HEADS, :],
                )
            if w > 1 and 0 < s <= NSUB and b == 0:
                h0 = (s - 1) * N_HEADS
                qe.dma_start(
                    out=RL[h0 : h0 + N_HEADS, SUB:SUBLEN],
                    in_=st[0:N_HEADS, 0 : w - 1],
                )
                nc.scalar.dma_start(
                    out=RV[h0 : h0 + N_HEADS, SUB:SUBLEN],
                    in_=st[N_HEADS : 2 * N_HEADS, 0 : w - 1],
                )

        # Groups are processed block-major: every subchunk's block 0 first,
        # then block 1, ... so each column range completes across ALL
        # partitions as early as possible. proc index p -> group id.
        def proc_to_group(p):
            return 4 * (p % NSUB) + p // NSUB if p < NSUB * GPS else p

        # After processing index pmax's probe flush, these chunks are ready.
        passb_after = {}
        for c, (a, e) in enumerate(CHUNKS):
            lastcol = e + w - 2
            if lastcol >= SUB:  # needs the (s-1 <- s) halo writes, incl. trailing
                pmax = NG - 1
            else:
                pmax = (lastcol // G) * NSUB + NSUB - 1
            pmax = min(pmax, NG - 1)
            # Stagger chunks that share a readiness point: a single oversized
            # DVE bundle delays evictions enough to starve the probe PSUM ring
            # and stall the PE.
            while pmax < NG - 1 and pmax in passb_after:
                pmax = min(pmax + 2, NG - 1)
            passb_after.setdefault(pmax, []).append(c)

        # ---------------- pass A: MLP + probes ----------------
        # Trailing halo-only group needs just w-1 tokens.
        LW = min(G, ((w - 1 + 63) // 64) * 64) if NG > GPC else G
        pending = None   # (g, yt, gw) awaiting probe matmul
        chunks_done = set()
        for p in range(NG):
            g = proc_to_group(p)
            gw = LW if p == NG - 1 and NG > GPC else G
            if p == 0:
                # Interleave the first x-group quarters with the w1 loads so
                # the PE's first matmul is gated on minimal DMA issues; b1/qv
                # (needed only by the later relu/probe) are issued from the
                # Scalar queue so they don't delay x prefetch on Sync.
                xt = xpool.tile([P, ND, G], MDT, tag="xt")
                nq = ND // 4
                for q4 in range(4):
                    nc.sync.dma_start(
                        out=xt[:, q4 * nq : (q4 + 1) * nq, :],
                        in_=xg[g, :, q4 * nq : (q4 + 1) * nq, :],
                    )
                    nc.sync.dma_start(
                        out=w1_sb[:, q4 * nq : (q4 + 1) * nq, :],
                        in_=w1p[:, q4 * nq : (q4 + 1) * nq, :],
                    )
                nc.scalar.dma_start(out=b1_sb[:], in_=b1p[:])
                nc.scalar.dma_start(out=qv_sb[:], in_=qvp[:])
            else:
                xt = xpool.tile([P, ND, gw], MDT, tag="xt")
                nc.sync.dma_start(out=xt[:], in_=xg[g, :, :, 0:gw])
            yt = ypool.tile([P, NH2, gw], MDT if DR is not None else qvp.dtype,
                            tag="yt")
            for hh in range(NH2):
                ypt = psy.tile([P, gw], F32, tag="ypsum")
                if DR is not None:
                    for dp in range(ND // 2):
                        nc.tensor.matmul(
                            ypt[:],
                            w1_sb[:, 2 * dp : 2 * dp + 2, hh * P : (hh + 1) * P],
                            xt[:, 2 * dp : 2 * dp + 2, :],
                            start=(dp == 0),
                            stop=(dp == ND // 2 - 1),
                            perf_mode=DR,
                        )
                else:
                    for d in range(ND):
                        nc.tensor.matmul(
                            ypt[:],
                            w1_sb[:, d, hh * P : (hh + 1) * P],
                            xt[:, d, :],
                            start=(d == 0),
                            stop=(d == ND - 1),
                        )
                nc.scalar.activation(
                    yt[:, hh, :], ypt[:], AF.Relu,
                    bias=b1_sb[:, hh : hh + 1], scale=1.0 / SCALE_W,
                )
            if pending is not None:
                gp, ytp, gwp = pending
                emit_probes(gp, ytp, gwp, last=(p >= NG - 1))
                for c in passb_after.get(p - 1, []):
                    emit_passb_chunk(c)
                    chunks_done.add(c)
            pending = (g, yt, gw)
        # Drain the software pipeline.
        gp, ytp, gwp = pending
        emit_probes(gp, ytp, gwp, last=True)
        for c in range(NCHK):
            if c not in chunks_done:
                emit_passb_chunk(c)

        # ---------------- final reduction + store ----------------
        res2 = bp.tile([P, 2], F32)
        nc.vector.reduce_max(out=res2[:, 0:1], in_=sm[:, 0:NCHK], axis=AX.X)
        nc.vector.reduce_max(
            out=res2[:, 1:2], in_=sm[:, NCHK : 2 * NCHK], axis=AX.X
        )
        nc.sync.dma_start(out=res[:], in_=res2[:])

    nc.compile()
    return nc


MM_DTYPE = "f8dr"


def _get_nc(w: int):
    key = (w, MM_DTYPE)
    nc = _NC_CACHE.get(key)
    if nc is None:
        nc = _build(w, MM_DTYPE)
        _NC_CACHE[key] = nc
    return nc


def _mm_cast(a: np.ndarray) -> np.ndarray:
    """Convert to the MLP matmul input dtype (host-side rounding)."""
    if MM_DTYPE == "f16":
        return a.astype(np.float16)
    if MM_DTYPE == "f8dr":
        import ml_dtypes

        return a.astype(ml_dtypes.float8_e4m3)
    if MM_DTYPE == "bf16":
        import ml_dtypes

        return a.astype(ml_dtypes.bfloat16)
    return _round_fp32r(a)


def _prep_inputs(x, w1, b1, queries, values, w):
    """Host-side packing: pad + round + transpose into DMA-friendly layouts.
    Returns the per-core in_maps for run_bass_kernel_spmd."""
    NG = -(-(TPC + w - 1) // G)
    NGG = (N_CORES - 1) * GPC + NG  # distinct global groups incl. final halo
    xpad = np.zeros((NGG * G, D_MODEL), dtype=np.float32)
    xpad[:N_TOKENS] = x
    xr = _mm_cast(xpad)
    # [gg, p, d, t] = xpad[gg*G + t, d*128 + p]
    xg_all = np.ascontiguousarray(
        xr.reshape(NGG, G, ND, P).transpose(0, 3, 2, 1)
    )
    w1p = np.ascontiguousarray(
        _mm_cast(w1 * SCALE_W).reshape(ND, P, D_HID).transpose(1, 0, 2)
    )
    b1p = np.ascontiguousarray(np.asarray(b1, np.float32).reshape(NH2, P).T)
    # Combined probe weights: [k, hh, m] with columns 0..15 = queries.T
    # chunk, columns 16..31 = values.T chunk.
    qv = np.concatenate(
        [np.asarray(queries, np.float32), np.asarray(values, np.float32)], axis=0
    )  # [32, 256]
    if MM_DTYPE == "f8dr":
        qvT = _mm_cast(qv * QV_SCALE).T.reshape(NH2, P, 2 * N_HEADS)
    else:
        qvT = qv.astype(np.float16).T.reshape(NH2, P, 2 * N_HEADS)  # [hh, k, m]
    qvp = np.ascontiguousarray(qvT.transpose(1, 0, 2))
    in_maps = []
    for c in range(N_CORES):
        in_maps.append(
            {
                "xg": xg_all[c * GPC : c * GPC + NG],
                "w1p": w1p,
                "b1p": b1p,
                "qvp": qvp,
            }
        )
    return in_maps


def _combine(results, w):
    """Host-side final reduction: per-core [128, 2] -> scalar."""
    qvs = QV_SCALE if MM_DTYPE == "f8dr" else 1.0
    best = np.full(N_HEADS, -np.inf, dtype=np.float64)
    for c in range(N_CORES):
        r = np.asarray(results[c]["res"], dtype=np.float64).reshape(NSUB, N_HEADS, 2)
        if c == N_CORES - 1 and w >= 2:
            r = r.copy()
            r[NSUB - 1, :, 1] = -np.inf  # windows past n - w on the last core
        best = np.maximum(best, r.max(axis=(0, 2)))
    return np.asarray((best / qvs).sum(), dtype=np.float32)


def kernel(x, w1, b1, queries, values, window_size):
    from concourse.bass_utils import run_bass_kernel_spmd

    x = np.asarray(x, dtype=np.float32)
    w1 = np.asarray(w1, dtype=np.float32)
    b1 = np.asarray(b1, dtype=np.float32)
    queries = np.asarray(queries, dtype=np.float32)
    values = np.asarray(values, dtype=np.float32)
    w = int(np.asarray(window_size))
    assert x.shape == (N_TOKENS, D_MODEL), x.shape
    assert 1 <= w <= G + 1  # halo duplication reads at most one group

    key = (w, MM_DTYPE)
    fresh = key not in _NC_CACHE
    nc = _get_nc(w)
    in_maps = _prep_inputs(x, w1, b1, queries, values, w)
    last_err = None
    for attempt in range(4):
        try:
            if fresh:
                # Warm-up run: the first execution after NEFF load has been
                # observed to race input upload; discard it.
                run_bass_kernel_spmd(nc, in_maps, core_ids=list(range(N_CORES)))
                fresh = False
            out = run_bass_kernel_spmd(nc, in_maps, core_ids=list(range(N_CORES)))
            return _combine(out.results, w)
        except Exception as e:  # transient terminal/device failures
            last_err = e
            import time as _time

            # Device-unrecoverable states have been observed to need ~60s.
            _time.sleep(15.0 * (attempt + 1))
    raise last_err


# Optional: expose a traced run for profiling from test harnesses.
def kernel_traced(x, w1, b1, queries, values, window_size, tmpdir=None):
    from concourse.bass_utils import run_bass_kernel_spmd

    w = int(np.asarray(window_size))
    nc = _get_nc(w)
    in_maps = _prep_inputs(
        np.asarray(x, np.float32),
        np.asarray(w1, np.float32),
        np.asarray(b1, np.float32),
        np.asarray(queries, np.float32),
        np.asarray(values, np.float32),
        w,
    )
    out = run_bass_kernel_spmd(
        nc, in_maps, core_ids=list(range(N_CORES)), trace=True, tmpdir=tmpdir
    )
    return _combine(out.results, w), out
